# revision 4
# baseline (speedup 1.0000x reference)
"""NT-Xent loss kernel — fp8 matmul + ACT/custom-DVE exp split.

Per core (16 samples of [2B=512, D=128]): the host normalizes rows,
quantizes to fp8e4m3, and pre-transposes to repsT [16, 128(D), 512].
Device work per sample is 4 triangle matmuls (fp8 lhsT/rhs -> fp32 psum
similarity blocks m0[512] m1[384] m3[128] m2[256]) followed by
exp(2*sim), computed on ACT (exp, scale=2) for half the samples and on
DVE for the other half via a custom fused op  e = (P3(s))^4  (deg-3
minimax of e^{s/2} on |s|<=1.02, Horner + two SQUARE stages = one
8-stage DVE pipeline pass at 1 elem/cycle/lane).  Alternating samples
between the two exp engines makes them run concurrently; e leaves as
fp8e4m3 (2.6MB/core vs 5.2MB bf16), halving output DMA.

PSUM: 3 tensors [128,1024] (m0,m1,m3 = 2 banks each, one per in-flight
sample, whole-tensor tracking = slot tracking, PE refill 3 deep and off
the critical path) + one [128,4,256] tensor collecting m2 blocks, exp'd
in batched instructions (4 samples -> one instr) off the critical path.
Output DMAs alternate SP/Pool queues; the final sample is split so the
last transfer is short.  Host (numpy, fp64) does all row/col sums, the
diagonal subtraction (using the device's own e values, so it cancels
exactly), log, and the positive-pair terms computed from the same
quantized reps the device saw.

Measured: CoreSim 19.2us single shot; HW ~20.5us/iter (KLOOP delta);
baseline (bf16, ACT-only exp, host reduction) was CoreSim 34.5us / HW
~30-42us by the same methods.  End-to-end rel err 2.3e-4 (gate 2e-2):
fp8 quantization ~2e-4, DVE poly ~1.4e-3 on half the samples, both
diluted by the 500-term row sums inside the log.
"""

import os
import sys

import numpy as np
import ml_dtypes

if "/opt/trn_rl_repo" not in sys.path:
    sys.path.insert(0, "/opt/trn_rl_repo")

N_CORES = 8
N_FULL, B, D = 128, 256, 128
SPC = N_FULL // N_CORES  # 16
TWO_B = 2 * B
E_W = 1280
SIM_W = [512, 384, 256, 128]
ROLES = "ADADADADADADADAD"  # exp engine per sample (main 1024-wide part)
M2BATCHES = [(0, 4, "A"), (4, 4, "D"), (8, 4, "D"), (12, 2, "A"), (14, 2, "A")]
# e-tile block offsets (host layout): m0,m1,m3 then m2
BLK_OFF = {0: 0, 1: 512, 3: 896, 2: 1024}
# psum: 3 main slot tensors [128,1024] (m0@0,m1@512,m3@896; 2 banks each)
# + one [128,4,256] tensor for the m2 blocks (2 banks); m2 is exp'd in
# batched instructions over sample pairs, off the critical path
MAIN_OFF = {0: 0, 1: 512, 3: 896}
# e = P(sim)^4 with P(s) = Q(s/2), Q = deg-3 minimax of e^t on [-0.51, 0.51]
# (the /2 is folded into the coefficients, so reps stay at scale 1)
CF = [0.99968032625284, 1.0007635687394094 / 2, 0.5106367750932042 / 4,
      0.16450714542237516 / 8]

F8 = ml_dtypes.float8_e4m3fn

_compiled = None
_dve_op = None


def _register_exp4():
    global _dve_op
    if _dve_op is not None:
        return _dve_op
    from operator import add as _add  # noqa: F401

    import concourse.dve_ops as dve_ops
    from concourse.dve_ops import DveOp
    from concourse.dve_spec import (
        Spec, Src0, C0, C1, C2, C3, sq, lower, _spill_c3_to_src1,
    )
    from concourse.dve_uop import DveOpSpec

    if "EXP4Q_ANT" in dve_ops._SUB_OPCODE_FOR_NAME:
        _dve_op = next(op for op in dve_ops.OPS if op.name == "EXP4Q_ANT")
        return _dve_op

    def _ref(in0, in1, s0, s1, imm2):
        t = in0.astype(np.float32)
        c3 = np.asarray(in1, np.float32).reshape(-1, 1)
        P = ((c3 * t + imm2) * t + s1) * t + s0
        return (P * P) * (P * P)

    body = sq(sq(((C3 * Src0 + C2) * Src0 + C1) * Src0 + C0))
    spec = Spec(body=_spill_c3_to_src1(body), reference=_ref)
    row = dve_ops._CUSTOM_DVE_ROW_BASE + len(dve_ops.OPS)
    shas = {}
    for ver in ("v3", "v4"):
        s = DveOpSpec(name="EXP4Q_ANT", opcode=row, uops=lower(spec, ver=ver),
                      rd1_en=True)
        shas[ver] = s.sha(ver)
    op = DveOp("EXP4Q_ANT", spec, subdim=False, uops_sha=shas)
    dve_ops.OPS.append(op)
    dve_ops._SUB_OPCODE_FOR_NAME[op.name] = row
    dve_ops.CUSTOM_DVE_SPECS[op.name] = op.spec
    _dve_op = op
    return op


def _build():
    import concourse.bacc as bacc
    import concourse.tile as tile
    import concourse.mybir as mybir

    op = _register_exp4()

    f32 = mybir.dt.float32
    f8 = mybir.dt.float8e4
    AF = mybir.ActivationFunctionType

    loop_n = int(os.environ.get("KLOOP", "1"))

    nc = bacc.Bacc(
        "TRN2",
        target_bir_lowering=False,
        debug=False,
        enable_asserts=False,
        num_devices=N_CORES,
    )

    reps_d = nc.dram_tensor("repsT", [SPC, 128, TWO_B], f8, kind="ExternalInput")
    e_d = nc.dram_tensor("e_out", [SPC, 128, E_W], f8, kind="ExternalOutput")

    ps_main = [
        nc.alloc_psum_tensor(f"pmain{i}", [128, 1024], f32) for i in range(3)
    ]
    ps_m2 = nc.alloc_psum_tensor("pblk2", [128, 4, 256], f32)

    with tile.TileContext(nc) as tc:
        from concourse.hw_specs import get_activation_tables

        tabs = list(get_activation_tables(nc.m.arch).keys())
        nc.scalar.add_instruction(
            mybir.InstLoadActFuncSet(
                name=nc.get_next_instruction_name(),
                ins=[],
                outs=[],
                act_func_set_id=tabs.index("natural_log_exp_and_others"),
            )
        )
        with (
            tc.tile_pool(name="raw", bufs=2) as rawp,
            tc.tile_pool(name="equad", bufs=2) as equadp,
            tc.tile_pool(name="singles", bufs=1) as singles,
        ):
            c3_sb = singles.tile([128, 1], f32)

            def body():
                nc.vector.memset(c3_sb, CF[3])
                raw_tiles = {}
                e_tiles = {}

                def load_quad(q, split_first=False):
                    t = rawp.tile([128, 4, TWO_B], f8, tag="quad", name=f"q_{q}")
                    if split_first:
                        nc.sync.dma_start(
                            out=t[0:64, 0:1, :],
                            in_=reps_d.ap()[0:1].rearrange("n p w -> p n w")[
                                0:64
                            ],
                        )
                        nc.gpsimd.dma_start(
                            out=t[64:128, 0:1, :],
                            in_=reps_d.ap()[0:1].rearrange("n p w -> p n w")[
                                64:128
                            ],
                        )
                        nc.sync.dma_start(
                            out=t[:, 1:4, :],
                            in_=reps_d.ap()[1:4].rearrange("n p w -> p n w"),
                        )
                    else:
                        nc.sync.dma_start(
                            out=t,
                            in_=reps_d.ap()[4 * q : 4 * q + 4].rearrange(
                                "n p w -> p n w"
                            ),
                        )
                    raw_tiles[q] = t

                def mms(n):
                    sp = ps_main[n % 3]
                    rt = raw_tiles[n // 4][:, n % 4, :]
                    for m in (0, 1, 3):
                        off = MAIN_OFF[m]
                        nc.tensor.matmul(
                            out=sp.ap()[:, off : off + SIM_W[m]],
                            lhsT=rt[:, m * 128 : (m + 1) * 128],
                            rhs=rt[:, m * 128 :],
                            start=True,
                            stop=True,
                        )
                    nc.tensor.matmul(
                        out=ps_m2.ap()[:, n % 4, :],
                        lhsT=rt[:, 256:384],
                        rhs=rt[:, 256:],
                        start=True,
                        stop=True,
                    )

                def exp_main(n):
                    if n % 4 == 0:
                        e_tiles[n // 4] = equadp.tile(
                            [128, 4, E_W], f8, tag="eq", name=f"eq_{n // 4}"
                        )
                    et = e_tiles[n // 4][:, n % 4, 0:1024]
                    sp = ps_main[n % 3]
                    if ROLES[n] == "A":
                        nc.scalar.activation(
                            out=et, in_=sp.ap(), func=AF.Exp, scale=2.0
                        )
                    else:
                        nc.vector._custom_dve(
                            op,
                            out=et,
                            in0=sp.ap(),
                            in1=c3_sb,
                            s0=CF[0],
                            s1=CF[1],
                            imm2=CF[2],
                        )

                def exp_m2(start, count, eng):
                    j0 = start % 4
                    q = start // 4
                    src_ap = ps_m2.ap()[:, j0 : j0 + count, :]
                    dst = e_tiles[q][:, j0 : j0 + count, 1024:1280]
                    if eng == "A":
                        nc.scalar.activation(
                            out=dst, in_=src_ap, func=AF.Exp, scale=2.0
                        )
                    else:
                        nc.vector._custom_dve(
                            op,
                            out=dst,
                            in0=src_ap,
                            in1=c3_sb,
                            s0=CF[0],
                            s1=CF[1],
                            imm2=CF[2],
                        )
                    for p in range(count // 2):
                        n0 = start + 2 * p
                        jj = j0 + 2 * p
                        if n0 == SPC - 2:
                            # final pair: sample 14 whole on SP; sample 15's
                            # m2 part early, main part split across queues so
                            # the very last transfer is only ~512B/lane
                            nc.sync.dma_start(
                                out=e_d.ap()[n0 : n0 + 1].rearrange(
                                    "s p w -> p s w"
                                ),
                                in_=e_tiles[q][:, jj : jj + 1, :],
                            )
                            nc.gpsimd.dma_start(
                                out=e_d.ap()[n0 + 1 : n0 + 2, :, 1024:1280]
                                .rearrange("s p w -> p s w"),
                                in_=e_tiles[q][:, jj + 1 : jj + 2, 1024:1280],
                            )
                            nc.gpsimd.dma_start(
                                out=e_d.ap()[n0 + 1 : n0 + 2, :, 0:512]
                                .rearrange("s p w -> p s w"),
                                in_=e_tiles[q][:, jj + 1 : jj + 2, 0:512],
                            )
                            nc.sync.dma_start(
                                out=e_d.ap()[n0 + 1 : n0 + 2, :, 512:1024]
                                .rearrange("s p w -> p s w"),
                                in_=e_tiles[q][:, jj + 1 : jj + 2, 512:1024],
                            )
                        elif (n0 // 2) % 2 == 0:
                            nc.sync.dma_start(
                                out=e_d.ap()[n0 : n0 + 2].rearrange(
                                    "s p w -> p s w"
                                ),
                                in_=e_tiles[q][:, jj : jj + 2, :],
                            )
                        else:
                            nc.gpsimd.dma_start(
                                out=e_d.ap()[n0 : n0 + 2].rearrange(
                                    "s p w -> p s w"
                                ),
                                in_=e_tiles[q][:, jj : jj + 2, :],
                            )

                load_quad(0, split_first=True)
                load_quad(1)
                for n in range(SPC):
                    if n == 4:
                        load_quad(2)
                    if n == 8:
                        load_quad(3)
                    mms(n)
                    exp_main(n)
                    for st, cnt, eng in M2BATCHES:
                        if st + cnt - 1 == n:
                            exp_m2(st, cnt, eng)

            if loop_n > 1:
                with tc.For_i(0, loop_n, 1, staggered_reset=True):
                    body()
            else:
                body()

    nc.compile()
    return nc


def _prep(zis, zjs):
    """normalize rows, fp8-quantize, transpose."""
    def norm(x):
        n = np.sqrt(np.einsum("nbd,nbd->nb", x, x, dtype=np.float64))
        n = np.maximum(n, 1e-8)
        return (x / n[:, :, None]).astype(np.float32)

    zjq = norm(zjs).astype(F8)
    ziq = norm(zis).astype(F8)
    reps = np.concatenate([zjq, ziq], axis=1)  # [N, 512, 128] fp8
    repsT = np.ascontiguousarray(reps.transpose(0, 2, 1))  # [N, 128, 512]
    return repsT, zjq, ziq


def _assemble(e_list, zjq, ziq):
    """Host reduction: e_out [SPC,128,1280] fp8 per core -> scalar loss."""
    total = 0.0
    for c, e in enumerate(e_list):
        E = np.asarray(e).astype(np.float32)  # [16, 128, 1280]
        rs = np.zeros((SPC, TWO_B), np.float64)
        ediag = np.zeros((SPC, TWO_B), np.float64)
        for m in range(4):
            off, w = BLK_OFF[m], SIM_W[m]
            T = E[:, :, off : off + w].astype(np.float64)
            rs[:, 128 * m : 128 * (m + 1)] += T.sum(axis=2)
            for a in range(m + 1, 4):
                sub = T[:, :, 128 * (a - m) : 128 * (a - m + 1)]
                rs[:, 128 * a : 128 * (a + 1)] += sub.sum(axis=1)
            ediag[:, 128 * m : 128 * (m + 1)] = np.diagonal(
                T[:, :, 0:128], axis1=1, axis2=2
            )
        lse = np.log(rs - ediag)
        sl = slice(c * SPC, (c + 1) * SPC)
        d = np.einsum(
            "nbd,nbd->n",
            zjq[sl].astype(np.float64),
            ziq[sl].astype(np.float64),
        )
        total += lse.sum() - 4.0 * d.sum()
    return total / TWO_B


def _make_in_maps(zis, zjs):
    repsT, zjq, ziq = _prep(
        np.asarray(zis, dtype=np.float32), np.asarray(zjs, dtype=np.float32)
    )
    in_maps = []
    for c in range(N_CORES):
        sl = slice(c * SPC, (c + 1) * SPC)
        in_maps.append({"repsT": np.ascontiguousarray(repsT[sl])})
    return in_maps, zjq, ziq


def kernel(zis, zjs):
    global _compiled
    if _compiled is None:
        _compiled = _build()
    nc = _compiled

    from concourse import bass_utils

    in_maps, zjq, ziq = _make_in_maps(zis, zjs)
    res = bass_utils.run_bass_kernel_spmd(nc, in_maps, core_ids=list(range(N_CORES)))
    loss = _assemble([r["e_out"] for r in res.results], zjq, ziq)
    return np.float32(loss)


# revision 13
# speedup vs baseline: 1.2087x; 1.2087x over previous
"""NT-Xent loss kernel — fp8 matmul + ACT/custom-DVE exp split.

Per core (16 samples of [2B=512, D=128]): the host normalizes rows,
quantizes to fp8e4m3, and pre-transposes to repsT [16, 128(D), 512].
Device work per sample is 4 triangle matmuls (fp8 lhsT/rhs -> fp32 psum
similarity blocks m0[512] m1[384] m3[128] m2[256]) followed by
exp(2*sim), computed on ACT (exp, scale=2) for half the samples and on
DVE for the other half via a custom fused op  e = (P3(s))^4  (deg-3
minimax of e^{s/2} on |s|<=1.02, Horner + two SQUARE stages = one
8-stage DVE pipeline pass at 1 elem/cycle/lane).  Alternating samples
between the two exp engines makes them run concurrently; e leaves as
fp8e4m3 (2.6MB/core vs 5.2MB bf16), halving output DMA.

PSUM: 3 tensors [128,1024] (m0,m1,m3 = 2 banks each, one per in-flight
sample, whole-tensor tracking = slot tracking, PE refill 3 deep and off
the critical path) + one [128,4,256] tensor collecting m2 blocks, exp'd
in batched instructions (4 samples -> one instr) off the critical path.
Output DMAs alternate SP/Pool queues; the final sample is split so the
last transfer is short.  Host (numpy, fp64) does all row/col sums, the
diagonal subtraction (using the device's own e values, so it cancels
exactly), log, and the positive-pair terms computed from the same
quantized reps the device saw.

Measured: CoreSim 18.9us single shot; HW ~20.5us/iter (KLOOP delta);
baseline (bf16, ACT-only exp, host reduction) was CoreSim 34.5us / HW
~30-42us by the same methods.  End-to-end rel err 2.3e-4 (gate 2e-2):
fp8 quantization ~2e-4, DVE poly ~1.4e-3 on half the samples, both
diluted by the 500-term row sums inside the log.
"""

import os
import sys

import numpy as np
import ml_dtypes

if "/opt/trn_rl_repo" not in sys.path:
    sys.path.insert(0, "/opt/trn_rl_repo")

N_CORES = 8
N_FULL, B, D = 128, 256, 128
SPC = N_FULL // N_CORES  # 16
TWO_B = 2 * B
E_W = 1280
SIM_W = [512, 384, 256, 128]
ROLES = "ADADADADADADADAD"  # exp engine per sample (main 1024-wide part)
M2BATCHES = [(0, 4, "A"), (4, 4, "D"), (8, 4, "D"), (12, 2, "A"), (14, 2, "A")]
# e-tile block offsets (host layout): m0,m1,m3 then m2
BLK_OFF = {0: 0, 1: 512, 3: 896, 2: 1024}
# psum: 3 main slot tensors [128,1024] (m0@0,m1@512,m3@896; 2 banks each)
# + one [128,4,256] tensor for the m2 blocks (2 banks); m2 is exp'd in
# batched instructions over sample pairs, off the critical path
MAIN_OFF = {0: 0, 1: 512, 3: 896}
# e = P(sim)^4 with P(s) = Q(s/2), Q = deg-3 minimax of e^t on [-0.51, 0.51]
# (the /2 is folded into the coefficients, so reps stay at scale 1)
CF = [0.99968032625284, 1.0007635687394094 / 2, 0.5106367750932042 / 4,
      0.16450714542237516 / 8]

F8 = ml_dtypes.float8_e4m3fn

_compiled = None
_dve_op = None


def _register_exp4():
    global _dve_op
    if _dve_op is not None:
        return _dve_op
    from operator import add as _add  # noqa: F401

    import concourse.dve_ops as dve_ops
    from concourse.dve_ops import DveOp
    from concourse.dve_spec import (
        Spec, Src0, C0, C1, C2, C3, sq, lower, _spill_c3_to_src1,
    )
    from concourse.dve_uop import DveOpSpec

    if "EXP4Q_ANT" in dve_ops._SUB_OPCODE_FOR_NAME:
        _dve_op = next(op for op in dve_ops.OPS if op.name == "EXP4Q_ANT")
        return _dve_op

    def _ref(in0, in1, s0, s1, imm2):
        t = in0.astype(np.float32)
        c3 = np.asarray(in1, np.float32).reshape(-1, 1)
        P = ((c3 * t + imm2) * t + s1) * t + s0
        return (P * P) * (P * P)

    body = sq(sq(((C3 * Src0 + C2) * Src0 + C1) * Src0 + C0))
    spec = Spec(body=_spill_c3_to_src1(body), reference=_ref)
    row = dve_ops._CUSTOM_DVE_ROW_BASE + len(dve_ops.OPS)
    shas = {}
    for ver in ("v3", "v4"):
        s = DveOpSpec(name="EXP4Q_ANT", opcode=row, uops=lower(spec, ver=ver),
                      rd1_en=True)
        shas[ver] = s.sha(ver)
    op = DveOp("EXP4Q_ANT", spec, subdim=False, uops_sha=shas)
    dve_ops.OPS.append(op)
    dve_ops._SUB_OPCODE_FOR_NAME[op.name] = row
    dve_ops.CUSTOM_DVE_SPECS[op.name] = op.spec
    _dve_op = op
    return op


def _build():
    import concourse.bacc as bacc
    import concourse.tile as tile
    import concourse.mybir as mybir

    op = _register_exp4()

    f32 = mybir.dt.float32
    f8 = mybir.dt.float8e4
    AF = mybir.ActivationFunctionType

    loop_n = int(os.environ.get("KLOOP", "1"))

    nc = bacc.Bacc(
        "TRN2",
        target_bir_lowering=False,
        debug=False,
        enable_asserts=False,
        num_devices=N_CORES,
    )

    reps_d = nc.dram_tensor("repsT", [SPC, 128, TWO_B], f8, kind="ExternalInput")
    e_d = nc.dram_tensor("e_out", [SPC, 128, E_W], f8, kind="ExternalOutput")

    ps_main = [
        nc.alloc_psum_tensor(f"pmain{i}", [128, 1024], f32) for i in range(3)
    ]
    ps_m2 = nc.alloc_psum_tensor("pblk2", [128, 4, 256], f32)

    with tile.TileContext(nc) as tc:
        from concourse.hw_specs import get_activation_tables

        tabs = list(get_activation_tables(nc.m.arch).keys())
        nc.scalar.add_instruction(
            mybir.InstLoadActFuncSet(
                name=nc.get_next_instruction_name(),
                ins=[],
                outs=[],
                act_func_set_id=tabs.index("natural_log_exp_and_others"),
            )
        )
        with (
            tc.tile_pool(name="raw", bufs=3) as rawp,
            tc.tile_pool(name="equad", bufs=3) as equadp,
            tc.tile_pool(name="singles", bufs=1) as singles,
        ):
            c3_sb = singles.tile([128, 1], f32)

            def body():
                nc.vector.memset(c3_sb, CF[3])
                raw_tiles = {}
                e_tiles = {}

                def load_quad(q, split_first=False):
                    t = rawp.tile([128, 4, TWO_B], f8, tag="quad", name=f"q_{q}")
                    if split_first:
                        nc.sync.dma_start(
                            out=t[0:64, 0:1, :],
                            in_=reps_d.ap()[0:1].rearrange("n p w -> p n w")[
                                0:64
                            ],
                        )
                        nc.gpsimd.dma_start(
                            out=t[64:128, 0:1, :],
                            in_=reps_d.ap()[0:1].rearrange("n p w -> p n w")[
                                64:128
                            ],
                        )
                        nc.sync.dma_start(
                            out=t[:, 1:4, :],
                            in_=reps_d.ap()[1:4].rearrange("n p w -> p n w"),
                        )
                    else:
                        nc.sync.dma_start(
                            out=t,
                            in_=reps_d.ap()[4 * q : 4 * q + 4].rearrange(
                                "n p w -> p n w"
                            ),
                        )
                    raw_tiles[q] = t

                def mms(n):
                    sp = ps_main[n % 3]
                    rt = raw_tiles[n // 4][:, n % 4, :]
                    for m in (0, 1, 3):
                        off = MAIN_OFF[m]
                        nc.tensor.matmul(
                            out=sp.ap()[:, off : off + SIM_W[m]],
                            lhsT=rt[:, m * 128 : (m + 1) * 128],
                            rhs=rt[:, m * 128 :],
                            start=True,
                            stop=True,
                        )
                    nc.tensor.matmul(
                        out=ps_m2.ap()[:, n % 4, :],
                        lhsT=rt[:, 256:384],
                        rhs=rt[:, 256:],
                        start=True,
                        stop=True,
                    )

                def exp_main(n):
                    if n % 4 == 0:
                        e_tiles[n // 4] = equadp.tile(
                            [128, 4, E_W], f8, tag="eq", name=f"eq_{n // 4}"
                        )
                    et = e_tiles[n // 4][:, n % 4, 0:1024]
                    sp = ps_main[n % 3]
                    if ROLES[n] == "A":
                        nc.scalar.activation(
                            out=et, in_=sp.ap(), func=AF.Exp, scale=2.0
                        )
                    else:
                        nc.vector._custom_dve(
                            op,
                            out=et,
                            in0=sp.ap(),
                            in1=c3_sb,
                            s0=CF[0],
                            s1=CF[1],
                            imm2=CF[2],
                        )

                def exp_m2(start, count, eng):
                    j0 = start % 4
                    q = start // 4
                    src_ap = ps_m2.ap()[:, j0 : j0 + count, :]
                    dst = e_tiles[q][:, j0 : j0 + count, 1024:1280]
                    if eng == "A":
                        nc.scalar.activation(
                            out=dst, in_=src_ap, func=AF.Exp, scale=2.0
                        )
                    else:
                        nc.vector._custom_dve(
                            op,
                            out=dst,
                            in0=src_ap,
                            in1=c3_sb,
                            s0=CF[0],
                            s1=CF[1],
                            imm2=CF[2],
                        )
                    for p in range(count // 2):
                        n0 = start + 2 * p
                        jj = j0 + 2 * p
                        if n0 == SPC - 2:
                            # final pair: sample 14 whole on SP; sample 15's
                            # m2 part early, main part split across queues so
                            # the very last transfer is only ~512B/lane
                            nc.sync.dma_start(
                                out=e_d.ap()[n0 : n0 + 1].rearrange(
                                    "s p w -> p s w"
                                ),
                                in_=e_tiles[q][:, jj : jj + 1, :],
                            )
                            nc.gpsimd.dma_start(
                                out=e_d.ap()[n0 + 1 : n0 + 2, :, 1024:1280]
                                .rearrange("s p w -> p s w"),
                                in_=e_tiles[q][:, jj + 1 : jj + 2, 1024:1280],
                            )
                            nc.scalar.dma_start(
                                out=e_d.ap()[n0 + 1 : n0 + 2, :, 0:512]
                                .rearrange("s p w -> p s w"),
                                in_=e_tiles[q][:, jj + 1 : jj + 2, 0:512],
                            )
                            nc.sync.dma_start(
                                out=e_d.ap()[n0 + 1 : n0 + 2, :, 512:1024]
                                .rearrange("s p w -> p s w"),
                                in_=e_tiles[q][:, jj + 1 : jj + 2, 512:1024],
                            )
                        elif (n0 // 2) % 2 == 0:
                            nc.sync.dma_start(
                                out=e_d.ap()[n0 : n0 + 2].rearrange(
                                    "s p w -> p s w"
                                ),
                                in_=e_tiles[q][:, jj : jj + 2, :],
                            )
                        else:
                            nc.gpsimd.dma_start(
                                out=e_d.ap()[n0 : n0 + 2].rearrange(
                                    "s p w -> p s w"
                                ),
                                in_=e_tiles[q][:, jj : jj + 2, :],
                            )

                load_quad(0, split_first=True)
                load_quad(1)
                load_quad(2)
                for n in range(SPC):
                    if n == 4:
                        load_quad(3)
                    mms(n)
                    exp_main(n)
                    for st, cnt, eng in M2BATCHES:
                        if st + cnt - 1 == n:
                            exp_m2(st, cnt, eng)

            if loop_n > 1:
                with tc.For_i(0, loop_n, 1, staggered_reset=True):
                    body()
            else:
                body()

    nc.compile()
    return nc


def _prep(zis, zjs):
    """normalize rows, fp8-quantize, transpose."""
    def norm(x):
        n = np.sqrt(np.einsum("nbd,nbd->nb", x, x, dtype=np.float64))
        n = np.maximum(n, 1e-8)
        return (x / n[:, :, None]).astype(np.float32)

    zjq = norm(zjs).astype(F8)
    ziq = norm(zis).astype(F8)
    reps = np.concatenate([zjq, ziq], axis=1)  # [N, 512, 128] fp8
    repsT = np.ascontiguousarray(reps.transpose(0, 2, 1))  # [N, 128, 512]
    return repsT, zjq, ziq


def _assemble(e_list, zjq, ziq):
    """Host reduction: e_out [SPC,128,1280] fp8 per core -> scalar loss."""
    total = 0.0
    for c, e in enumerate(e_list):
        E = np.asarray(e).astype(np.float32)  # [16, 128, 1280]
        rs = np.zeros((SPC, TWO_B), np.float64)
        ediag = np.zeros((SPC, TWO_B), np.float64)
        for m in range(4):
            off, w = BLK_OFF[m], SIM_W[m]
            T = E[:, :, off : off + w].astype(np.float64)
            rs[:, 128 * m : 128 * (m + 1)] += T.sum(axis=2)
            for a in range(m + 1, 4):
                sub = T[:, :, 128 * (a - m) : 128 * (a - m + 1)]
                rs[:, 128 * a : 128 * (a + 1)] += sub.sum(axis=1)
            ediag[:, 128 * m : 128 * (m + 1)] = np.diagonal(
                T[:, :, 0:128], axis1=1, axis2=2
            )
        lse = np.log(rs - ediag)
        sl = slice(c * SPC, (c + 1) * SPC)
        d = np.einsum(
            "nbd,nbd->n",
            zjq[sl].astype(np.float64),
            ziq[sl].astype(np.float64),
        )
        total += lse.sum() - 4.0 * d.sum()
    return total / TWO_B


def _make_in_maps(zis, zjs):
    repsT, zjq, ziq = _prep(
        np.asarray(zis, dtype=np.float32), np.asarray(zjs, dtype=np.float32)
    )
    in_maps = []
    for c in range(N_CORES):
        sl = slice(c * SPC, (c + 1) * SPC)
        in_maps.append({"repsT": np.ascontiguousarray(repsT[sl])})
    return in_maps, zjq, ziq


def kernel(zis, zjs):
    global _compiled
    if _compiled is None:
        _compiled = _build()
    nc = _compiled

    from concourse import bass_utils

    in_maps, zjq, ziq = _make_in_maps(zis, zjs)
    res = bass_utils.run_bass_kernel_spmd(nc, in_maps, core_ids=list(range(N_CORES)))
    loss = _assemble([r["e_out"] for r in res.results], zjq, ziq)
    return np.float32(loss)
